# revision 10
# baseline (speedup 1.0000x reference)
"""Trainium2 Bass kernel for nn_Contrast_loss (B=8192, D=256, 100 classes).

Math: with mask = -same + 0.5*(1-same) + I and same_ii = 1,
    loss = 0.5*||s||^2 - 1.5*sum_c ||g_c||^2 + sum_i ||f_i||^2
where s = sum_i f_i and g_c = sum_{i: label_i = c} f_i.

Every term decomposes over feature columns, so feat is sharded column-wise
across the 8 cores (32 columns each); the host sums the per-core partials.
No cross-core collective is needed.

Per core, everything runs through one fp8 DoubleRow matmul stream:
  - the host re-encodes label as a one-hot fp8 matrix (exact in fp8) with an
    extra all-ones column (computes s in the same matmul), and feat as an
    fp8 hi/lo pair (hi = e4m3(f), lo = e4m3(f - hi); ~8-bit mantissa total).
  - the PE accumulates G = [onehot|1]^T @ [hi|lo] over 64 row chunks, two
    chunks per DoubleRow matmul.
  - the diag term sum ||f_i||^2 = sum hi^2 + sum lo^2 (the 2*hi*lo cross term
    is ~2e-5 of the total, dropped) comes from Scalar square-accumulate
    passes over fhl, overlapped with the matmul stream.
  - tail: q_c = ||g_c||^2 on DVE reading PSUM directly, weighted and summed
    with the diag partials into comb[128,1], which is DMA'd out; the host
    finishes the 128-lane + cross-core reduction.
"""

import numpy as np
import ml_dtypes

import concourse.bacc as bacc
import concourse.bass as bass
import concourse.mybir as mybir
import concourse.tile as tile
from concourse import bass_utils

B = 8192
D = 256
N_CORES = 8
DPC = D // N_CORES          # 32 feature columns per core
P = 128                     # partitions
CHUNKS = B // P             # 64 row chunks of 128
N_GROUPS = 4                # DMA / pipeline groups
CPG = CHUNKS // N_GROUPS    # 16 chunks per group
NCLS = 100                  # label values 0..99
NR = NCLS + 12              # one-hot cols + ones col + pad to mult-of-16 (dual-fp8 LDW)
LAMDA = 0.5

FP32 = mybir.dt.float32
BF16 = mybir.dt.bfloat16
FP8 = mybir.dt.float8e4
E4M3 = ml_dtypes.float8_e4m3

_CACHED_NC = None


def _build_nc():
    nc = bacc.Bacc("TRN2", target_bir_lowering=False, debug=False,
                   num_devices=N_CORES)

    oh_d = nc.dram_tensor("oh", [B, NR], FP8, kind="ExternalInput")
    fhl_d = nc.dram_tensor("fhl", [B, 2 * DPC], FP8, kind="ExternalInput")
    w_d = nc.dram_tensor("wv", [P, 1], FP32, kind="ExternalInput")
    out_d = nc.dram_tensor("out", [1, 1], FP32, kind="ExternalOutput")

    with tile.TileContext(nc) as tc:
        with (
            tc.tile_pool(name="big", bufs=1) as big,
            tc.tile_pool(name="small", bufs=1) as small,
            tc.tile_pool(name="psum", bufs=1, space="PSUM") as psum,
        ):
            # Row r = p*CHUNKS + k lives at (partition p, chunk k).
            oh_t = big.tile([P, CHUNKS, NR], FP8)
            fhl_t = big.tile([P, CHUNKS, 2 * DPC], FP8)
            sq_t = big.tile([P, CHUNKS, 2 * DPC], BF16)
            dacc = small.tile([P, N_GROUPS], FP32)
            w_t = small.tile([P, 1], FP32)
            qq = small.tile([P, 1], FP32)

            psum_g = psum.tile([NR, 2 * DPC], FP32)

            nc.scalar.dma_start(w_t[:], w_d.rearrange("p c -> p c"))
            nc.vector.memset(qq[:], 0.0)

            oh_src = oh_d.rearrange("(p k) c -> p k c", p=P)
            fhl_src = fhl_d.rearrange("(p k) d -> p k d", p=P)
            for g in range(N_GROUPS):
                ksl = slice(g * CPG, (g + 1) * CPG)
                ksl_a = slice(g * CPG, g * CPG + CPG // 2)
                ksl_b = slice(g * CPG + CPG // 2, (g + 1) * CPG)
                nc.sync.dma_start(oh_t[:, ksl_a, :], oh_src[:, ksl_a, :])
                nc.scalar.dma_start(oh_t[:, ksl_b, :], oh_src[:, ksl_b, :])
                nc.gpsimd.dma_start(fhl_t[:, ksl, :], fhl_src[:, ksl, :])
                # diag partials on Scalar (overlapped with the PE stream)
                nc.scalar.activation(sq_t[:, ksl, :], fhl_t[:, ksl, :],
                                     mybir.ActivationFunctionType.Square,
                                     accum_out=dacc[:, g:g + 1])
                for k in range(g * CPG, (g + 1) * CPG, 2):
                    nc.tensor.matmul(psum_g[:], oh_t[:, k:k + 2, :],
                                     fhl_t[:, k:k + 2, :],
                                     start=(k == 0), stop=(k == CHUNKS - 2),
                                     perf_mode=mybir.MatmulPerfMode.DoubleRow)

            # G rows: 0..99 = [g_hi | g_lo] per class, 100 = [s_hi | s_lo]
            gh = small.tile([NR, DPC], FP32)
            nc.vector.tensor_copy(gh[:], psum_g[:, 0:DPC])
            gt = small.tile([NR, DPC], FP32)
            nc.vector.tensor_add(gt[:], gh[:], psum_g[:, DPC:2 * DPC])
            qsc = small.tile([NR, DPC], FP32)
            nc.vector.tensor_mul(qsc[:], gt[:], gt[:])
            nc.vector.tensor_reduce(qq[0:NR, 0:1], qsc[:],
                                    mybir.AxisListType.X, mybir.AluOpType.add)
            dsum = small.tile([P, 1], FP32)
            nc.vector.tensor_reduce(dsum[:], dacc[:], mybir.AxisListType.X,
                                    mybir.AluOpType.add)
            comb = small.tile([P, 1], FP32)
            nc.vector.tensor_mul(comb[:], qq[:], w_t[:])
            nc.vector.tensor_add(comb[:], comb[:], dsum[:])
            res_t = small.tile([1, 1], FP32)
            nc.gpsimd.tensor_reduce(res_t[:], comb[:], mybir.AxisListType.C,
                                    mybir.AluOpType.add)
            nc.sync.dma_start(out_d[:], res_t[:])

    nc.compile()
    return nc


def _get_nc():
    global _CACHED_NC
    if _CACHED_NC is None:
        _CACHED_NC = _build_nc()
    return _CACHED_NC


def make_in_maps(feat, label):
    feat = np.asarray(feat, dtype=np.float32)
    lab = np.asarray(label).astype(np.int32)
    oh = (lab[:, None] == np.arange(NR, dtype=np.int32)[None, :])
    oh = oh.astype(E4M3)
    oh[:, NCLS] = E4M3(1.0)            # ones column -> s row
    hi = feat.astype(E4M3)
    lo = (feat - hi.astype(np.float32)).astype(E4M3)
    w = np.zeros((P, 1), dtype=np.float32)
    w[0:NCLS, 0] = -(1.0 + LAMDA)
    w[NCLS, 0] = LAMDA
    maps = []
    for m in range(N_CORES):
        csl = slice(m * DPC, (m + 1) * DPC)
        fhl = np.concatenate([hi[:, csl], lo[:, csl]], axis=1)
        maps.append({"oh": oh, "fhl": np.ascontiguousarray(fhl), "wv": w})
    return maps


def kernel(feat, label, _trace=False):
    nc = _get_nc()
    in_maps = make_in_maps(feat, label)
    res = bass_utils.run_bass_kernel_spmd(
        nc, in_maps, core_ids=list(range(N_CORES)), trace=_trace)
    total = np.float64(0.0)
    for r in res.results:
        total += np.float64(r["out"]).sum()
    out = np.float32(total)
    if _trace:
        return out, res
    return out
